# revision 18
# baseline (speedup 1.0000x reference)
"""Ernie4.5 MoE sparse block on 8 trn2 NeuronCores.

Strategy (expert-parallel, dense over tokens):
  - 16 experts sharded 2/core; each core computes its 2 experts for ALL
    2048 tokens in bf16, with router combine-weights folded into the
    activations before the down-projection (so unselected experts
    contribute exactly 0).
  - Shared-expert MLP is tensor-parallel along its intermediate dim
    (128 slice / core) and accumulated in the same PSUM group as the
    expert down-projections.
  - Router (logits, softmax-free top-4, combine weights) is computed in
    fp32 on every core (cheap, and fp32 selection is required: bf16
    logits flip ~5% of top-4 picks).
  - One bf16 ReduceScatter sums the 8 per-core partials; core r ends up
    with tokens [256r:256(r+1)] which the host concatenates.

Host-side work is marshalling only: reshape/transpose/cast of inputs,
concatenation of outputs. All arithmetic runs on device.
"""
import os
import sys

sys.path.insert(0, "/opt/trn_rl_repo")

import numpy as np
import ml_dtypes

from concourse import bass, bacc, mybir, tile
from concourse import bass_utils
from concourse import masks

P = 128
B, S, H = 2, 1024, 1024
T = B * S                    # 2048 tokens
E, I, K = 16, 512, 4
SI = 1024                    # shared intermediate (512*2)
N_CORES = 8
ELOC = E // N_CORES          # 2 experts per core
SIL = SI // N_CORES          # 128 shared-intermediate slice per core
HT = H // P                  # 8 h-tiles
IT = I // P                  # 4 i-tiles
TT = T // P                  # 16 token-tiles
NCH = 512                    # matmul moving free dim
TC = T // NCH                # 4 token chunks
HC = H // NCH                # 2 h chunks
TLOC = T // N_CORES          # 256 tokens per core after RS
CAP = 640                    # per-expert compact token capacity
CT = CAP // P                # 5 compact tiles

f32 = mybir.dt.float32
bf16 = mybir.dt.bfloat16
bfnp = ml_dtypes.bfloat16

PHASE = int(os.environ.get("KERNEL_PHASE", "99"))
DEBUG = int(os.environ.get("KERNEL_DEBUG", "0"))


def _emit(nc, tc, pp, sp, wp, ps, dram, tensors):
    (xT_d, gwr_d, cb_d, sel_d, w1_d, d3_d, sg1_d, sd3_d, out_d, lg_d,
     x_bf_d) = tensors

    # ---------- constants ----------
    ident = pp.tile([P, P], f32, tag="ident")
    masks.make_identity(nc, ident[:])

    cbb = pp.tile([P, E], f32, tag="cbb")       # corr_bias bcast
    ones1 = pp.tile([1, P], f32, tag="ones1")
    nc.gpsimd.memset(ones1[:], 1.0)
    cb_sb = pp.tile([1, E], f32, tag="cb_sb")
    nc.sync.dma_start(cb_sb[:], cb_d[:])
    ps_cb = ps.tile([P, NCH], f32, tag="ps")
    nc.tensor.matmul(ps_cb[:, :E], ones1[:], cb_sb[:],
                     start=True, stop=True)
    nc.vector.tensor_copy(cbb[:], ps_cb[:, :E])

    # ---------- router matmuls (fp32) + x cast ----------
    xTb = [pp.tile([P, T], bf16, tag=f"xTb{h}", name=f"xTb{h}")
           for h in range(HT)]
    ps_lg = ps.tile([P, TT * E], f32, tag="pslg", bufs=1)
    nc.vector.memset(ps_lg[:], 0.0)
    for h in range(HT):
        xf = sp.tile([P, T], f32, tag="xf")
        nc.sync.dma_start(xf[:], xT_d[h])
        nc.scalar.activation(xTb[h][:], xf[:],
                             mybir.ActivationFunctionType.Copy)
        gr = sp.tile([P, E], f32, tag="gr")
        nc.sync.dma_start(gr[:], gwr_d[h])
        for t in range(TT):
            nc.tensor.matmul(
                ps_lg[:, t * E:(t + 1) * E],
                xf[:, t * P:(t + 1) * P],
                gr[:],
                start=False, stop=(h == HT - 1),
                skip_group_check=True)

    lg_all = pp.tile([P, TT * E], f32, tag="lg_all")
    nc.vector.tensor_copy(lg_all[:], ps_lg[:])
    # logits output: row (t*128+p) of lg_d = lg_all[p, t*16:(t+1)*16]
    for t in range(TT):
        nc.sync.dma_start(lg_d[t * P:(t + 1) * P, :],
                          lg_all[:, t * E:(t + 1) * E])
    if PHASE < 2:
        return

    # ---------- routing (softmax-free top-4 combine weights) ----------
    cw_all = pp.tile([P, TT * E], f32, tag="cw_all")
    for t in range(TT):
        Lt = lg_all[:, t * E:(t + 1) * E]
        m8 = sp.tile([P, 8], f32, tag="m8")
        nc.vector.max(m8[:], Lt)
        negm = sp.tile([P, 1], f32, tag="negm")
        nc.vector.tensor_scalar_mul(negm[:], m8[:, 0:1], -1.0)
        pun = sp.tile([P, E], f32, tag="pun")
        nc.scalar.activation(pun[:], Lt,
                             mybir.ActivationFunctionType.Exp,
                             bias=negm[:])
        ssum = sp.tile([P, 1], f32, tag="ssum")
        nc.vector.reduce_sum(ssum[:], pun[:], axis=mybir.AxisListType.X)
        # selection scores s = pun + cb*ssum (monotone-equiv to
        # softmax(logits)+cb per token)
        s_un = sp.tile([P, E], f32, tag="s_un")
        nc.vector.scalar_tensor_tensor(
            s_un[:], cbb[:], ssum[:, 0:1], pun[:],
            op0=mybir.AluOpType.mult, op1=mybir.AluOpType.add)
        s8 = sp.tile([P, 8], f32, tag="s8")
        nc.vector.max(s8[:], s_un[:])
        # rw = (s >= 4th-largest) * pun
        rw = sp.tile([P, E], f32, tag="rw")
        nc.vector.scalar_tensor_tensor(
            rw[:], s_un[:], s8[:, K - 1:K], pun[:],
            op0=mybir.AluOpType.is_ge, op1=mybir.AluOpType.mult)
        rsum = sp.tile([P, 1], f32, tag="rsum")
        nc.vector.reduce_sum(rsum[:], rw[:], axis=mybir.AxisListType.X)
        zeps = sp.tile([P, 1], f32, tag="zeps")
        nc.vector.tensor_scalar_mul(zeps[:], ssum[:], 1e-12)
        den = sp.tile([P, 1], f32, tag="den")
        nc.vector.tensor_tensor(den[:], rsum[:], zeps[:],
                                op=mybir.AluOpType.max)
        rden = sp.tile([P, 1], f32, tag="rden")
        nc.vector.reciprocal(rden[:], den[:])
        nc.vector.tensor_scalar_mul(
            cw_all[:, t * E:(t + 1) * E], rw[:], rden[:, 0:1])
    if PHASE < 3:
        return

    # ---------- shared expert stage 1 (si-slice = 128, dense tokens) ----
    # Emitted early: independent of routing, keeps PE busy while the
    # routing DVE chain and the compaction DMA chains run.
    ssg = pp.tile([P, T], bf16, tag="ssg")
    saT = pp.tile([P, T], bf16, tag="saT")
    for pj in range(2):
        w8 = wp.tile([P, HT * P], bf16, tag="w8", name="w8s")
        for h in range(HT):
            nc.sync.dma_start(w8[:, h * P:(h + 1) * P], sg1_d[pj, h])
        pst = [ps.tile([P, NCH], f32, tag="ps", name="psts")
               for _ in range(TC)]
        for h in range(HT):
            for tch in range(TC):
                nc.tensor.matmul(
                    pst[tch],
                    w8[:, h * P:(h + 1) * P],
                    xTb[h][:, tch * NCH:(tch + 1) * NCH],
                    start=(h == 0), stop=(h == HT - 1))
        for tch in range(TC):
            sl = slice(tch * NCH, (tch + 1) * NCH)
            if pj == 0:
                nc.scalar.activation(ssg[:, sl], pst[tch][:],
                                     mybir.ActivationFunctionType.Silu)
            else:
                nc.vector.tensor_tensor(saT[:, sl], pst[tch][:],
                                        ssg[:, sl],
                                        op=mybir.AluOpType.mult)
    if PHASE < 4:
        return

    # ---------- cw transpose -> cwT [16, T] f32 ----------
    cwT = pp.tile([E, T], f32, tag="cwT")
    for t in range(TT):
        ps_tr = ps.tile([P, P], f32, tag="ps", name="ps_tr")
        nc.tensor.transpose(ps_tr[:E, :],
                            cw_all[:, t * E:(t + 1) * E], ident[:])
        nc.vector.tensor_copy(cwT[:, t * P:(t + 1) * P], ps_tr[:E, :])

    # ---------- compaction maps (per local expert) ----------
    # cwrows [2, T] = cw rows of this core's 2 experts (via one-hot matmul)
    sel2 = pp.tile([E, ELOC], f32, tag="sel2")
    for j in range(ELOC):
        nc.sync.dma_start(sel2[:, j:j + 1], sel_d[j, :, 0:1])
    # stackT rows: 0-1 cw value, 2-3 scatter offset, 4-5 gather position.
    # DVE can only write at partition base 0, so rows 2-5 are filled via
    # SBUF->SBUF DMA from base-0 temp tiles.
    stackT = pp.tile([8, T], f32, tag="stackT")
    t1 = pp.tile([ELOC, T], f32, tag="scan_t1")
    t2 = pp.tile([ELOC, T], f32, tag="scan_t2")
    t3 = pp.tile([ELOC, T], f32, tag="scan_t3")
    cwr = stackT[0:ELOC, :]
    for tch in range(TC):
        ps_r = ps.tile([ELOC, NCH], f32, tag="ps", name="ps_r")
        nc.tensor.matmul(ps_r[:], sel2[:],
                         cwT[:, tch * NCH:(tch + 1) * NCH],
                         start=True, stop=True)
        nc.vector.tensor_copy(cwr[:, tch * NCH:(tch + 1) * NCH], ps_r[:])
    BIG = 1.0e6
    # t1 = mask; t2 = inclusive-cumsum - 1; t1 = notmask (reuse)
    nc.vector.tensor_scalar(t1[:], cwr, 0.0, None,
                            op0=mybir.AluOpType.is_gt)
    nc.vector.tensor_tensor_scan(t2[:], t1[:], t1[:], 0.0,
                                 op0=mybir.AluOpType.add,
                                 op1=mybir.AluOpType.bypass)
    nc.vector.tensor_scalar_add(t2[:], t2[:], -1.0)
    nc.vector.tensor_scalar(t1[:], cwr, 0.0, None,
                            op0=mybir.AluOpType.is_le)
    # scatter offset = pos-1 + notmask*BIG -> rows 2-3
    nc.vector.scalar_tensor_tensor(t3[:], t1[:], BIG, t2[:],
                                   op0=mybir.AluOpType.mult,
                                   op1=mybir.AluOpType.add)
    nc.sync.dma_start(stackT[ELOC:2 * ELOC, :], t3[:])
    # gather position = min(pos-1 + notmask*(CAP-1-(pos-1)), CAP-1) -> 4-5
    nc.vector.tensor_scalar(t3[:], t2[:], -1.0, float(CAP - 1),
                            op0=mybir.AluOpType.mult,
                            op1=mybir.AluOpType.add)
    nc.vector.tensor_tensor(t3[:], t1[:], t3[:], op=mybir.AluOpType.mult)
    nc.vector.tensor_tensor(t3[:], t3[:], t2[:], op=mybir.AluOpType.add)
    nc.vector.tensor_scalar_min(t3[:], t3[:], float(CAP - 1))
    nc.sync.dma_start(stackT[2 * ELOC:3 * ELOC, :], t3[:])
    # transpose per t-tile into [128, 8]; int-cast offset/pos columns
    trs = pp.tile([P, 8 * TT], f32, tag="trs")
    offi = pp.tile([P, ELOC * TT], mybir.dt.int32, tag="offi")
    posi = pp.tile([P, ELOC * TT], mybir.dt.int32, tag="posi")
    for t in range(TT):
        ps_t2 = ps.tile([P, P], f32, tag="ps", name="ps_t2")
        nc.tensor.transpose(ps_t2[:, :8],
                            stackT[:, t * P:(t + 1) * P], ident[:8, :8])
        nc.vector.tensor_copy(trs[:, t * 8:(t + 1) * 8], ps_t2[:, :8])
        nc.vector.tensor_copy(offi[:, t * ELOC:(t + 1) * ELOC],
                              trs[:, t * 8 + ELOC:t * 8 + 2 * ELOC])
        nc.vector.tensor_copy(posi[:, t * ELOC:(t + 1) * ELOC],
                              trs[:, t * 8 + 2 * ELOC:t * 8 + 3 * ELOC])

    # ---------- build compact (idx, cw) tables via masked scatter ------
    iota_t = pp.tile([P, TT], mybir.dt.int32, tag="iota_t")
    nc.gpsimd.iota(iota_t[:], pattern=[[P, TT]], base=0,
                   channel_multiplier=1)
    zi_sb = pp.tile([1, CAP], mybir.dt.int32, tag="zi_sb")
    nc.vector.memset(zi_sb[:], 0)
    zf_sb = pp.tile([1, CAP], f32, tag="zf_sb")
    nc.vector.memset(zf_sb[:], 0.0)
    idxC_d = [dram.tile([CAP, 1], mybir.dt.int32, name=f"idxC{j}")
              for j in range(ELOC)]
    cwC_d = [dram.tile([CAP, 1], f32, name=f"cwC{j}") for j in range(ELOC)]
    for j in range(ELOC):
        nc.sync.dma_start(idxC_d[j][:], zi_sb[:])
        nc.sync.dma_start(cwC_d[j][:], zf_sb[:])
        for t in range(TT):
            off_ap = bass.IndirectOffsetOnAxis(
                ap=offi[:, t * ELOC + j:t * ELOC + j + 1], axis=0)
            nc.gpsimd.indirect_dma_start(
                out=idxC_d[j][:], out_offset=off_ap,
                in_=iota_t[:, t:t + 1], in_offset=None,
                bounds_check=CAP - 1, oob_is_err=False)
            nc.gpsimd.indirect_dma_start(
                out=cwC_d[j][:],
                out_offset=bass.IndirectOffsetOnAxis(
                    ap=offi[:, t * ELOC + j:t * ELOC + j + 1], axis=0),
                in_=trs[:, t * 8 + j:t * 8 + j + 1], in_offset=None,
                bounds_check=CAP - 1, oob_is_err=False)

    # ---------- gather x rows into compact [CAP, H] then transpose -----
    xC_d2 = [dram.tile([CAP, H], bf16, name=f"xC{j}") for j in range(ELOC)]
    for j in range(ELOC):
        for ct in range(CT):
            idx_sb = sp.tile([P, 1], mybir.dt.int32, tag="idx_sb",
                             name="idx_sb")
            nc.sync.dma_start(idx_sb[:], idxC_d[j][ct * P:(ct + 1) * P, :])
            xg = sp.tile([P, H], bf16, tag="xg", name="xg")
            nc.gpsimd.indirect_dma_start(
                out=xg[:], out_offset=None,
                in_=x_bf_d[:],
                in_offset=bass.IndirectOffsetOnAxis(ap=idx_sb[:], axis=0))
            nc.sync.dma_start(xC_d2[j][ct * P:(ct + 1) * P, :], xg[:])
    xCT = [[pp.tile([P, CAP], bf16, tag=f"xCT{j}_{h}", name=f"xCT{j}_{h}")
            for h in range(HT)] for j in range(ELOC)]
    for j in range(ELOC):
        for h in range(HT):
            nc.sync.dma_start_transpose(
                xCT[j][h][:], xC_d2[j][:, h * P:(h + 1) * P])

    # ---------- compact cw broadcast [128, CAP] bf16 ----------
    CCH = [(0, NCH), (NCH, CAP - NCH)]     # free-dim chunks of CAP
    cwbC = [pp.tile([P, CAP], bf16, tag=f"cwbC{j}", name=f"cwbC{j}")
            for j in range(ELOC)]
    for j in range(ELOC):
        cwC_sb = sp.tile([1, CAP], f32, tag="cwC_sb", name="cwC_sb")
        nc.sync.dma_start(cwC_sb[:], cwC_d[j][:])
        for c0, cn in CCH:
            ps_b = ps.tile([P, NCH], f32, tag="ps", name="ps_b")
            nc.tensor.matmul(ps_b[:, :cn], ones1[:],
                             cwC_sb[:, c0:c0 + cn], start=True, stop=True)
            nc.vector.tensor_copy(cwbC[j][:, c0:c0 + cn], ps_b[:, :cn])
    if PHASE < 5:
        return

    # ---------- sparse stage 1+2 per (expert, proj, i-tile) ----------
    silugC = [pp.tile([P, CAP], bf16, tag=f"silugC{i}", name=f"silugC{i}")
              for i in range(IT)]
    aC = [[pp.tile([P, CAP], bf16, tag=f"aC{j}_{i}", name=f"aC{j}_{i}")
           for i in range(IT)] for j in range(ELOC)]
    for j in range(ELOC):
        for i in range(IT):
            for pj in range(2):
                w8 = wp.tile([P, HT * P], bf16, tag="w8", name="w8")
                for h in range(HT):
                    nc.sync.dma_start(w8[:, h * P:(h + 1) * P],
                                      w1_d[j, pj, i, h])
                pst = [ps.tile([P, NCH], f32, tag="ps", name="pst")
                       for _ in range(len(CCH))]
                for h in range(HT):
                    for ci, (c0, cn) in enumerate(CCH):
                        nc.tensor.matmul(
                            pst[ci][:, :cn],
                            w8[:, h * P:(h + 1) * P],
                            xCT[j][h][:, c0:c0 + cn],
                            start=(h == 0), stop=(h == HT - 1))
                for ci, (c0, cn) in enumerate(CCH):
                    sl = slice(c0, c0 + cn)
                    if pj == 0:
                        nc.scalar.activation(
                            silugC[i][:, sl], pst[ci][:, :cn],
                            mybir.ActivationFunctionType.Silu)
                    else:
                        tt = sp.tile([P, NCH], bf16, tag="tmpmul",
                                     name="tmpmul")
                        nc.vector.tensor_tensor(
                            tt[:, :cn], pst[ci][:, :cn], silugC[i][:, sl],
                            op=mybir.AluOpType.mult)
                        nc.vector.tensor_tensor(
                            aC[j][i][:, sl], tt[:, :cn], cwbC[j][:, sl],
                            op=mybir.AluOpType.mult)
    if PHASE < 6:
        return

    # ---------- weights for stage 3 ----------
    dwt = [[[pp.tile([P, NCH], bf16, tag=f"dw{j}_{i}_{hc}",
                     name=f"dw{j}_{i}_{hc}")
             for hc in range(HC)] for i in range(IT)] for j in range(ELOC)]
    sdwt = [pp.tile([P, NCH], bf16, tag=f"sdw{hc}", name=f"sdw{hc}")
            for hc in range(HC)]
    for j in range(ELOC):
        for i in range(IT):
            for hc in range(HC):
                nc.sync.dma_start(dwt[j][i][hc][:], d3_d[j, i, hc])
    for hc in range(HC):
        nc.sync.dma_start(sdwt[hc][:], sd3_d[hc])

    # ---------- shared stage 3 (dense) -> DRAM staging ----------
    shpart_d = dram.tile([T, H], bf16, name="shpart_d")
    for t in range(TT):
        ps3 = [ps.tile([P, NCH], f32, tag="ps", name="ps3s")
               for _ in range(HC)]
        outS = sp.tile([P, H], bf16, tag="outC", name="outS")
        for hc in range(HC):
            nc.tensor.matmul(ps3[hc][:], saT[:, t * P:(t + 1) * P],
                             sdwt[hc][:], start=True, stop=True)
            nc.vector.tensor_copy(outS[:, hc * NCH:(hc + 1) * NCH],
                                  ps3[hc][:])
        nc.sync.dma_start(shpart_d[t * P:(t + 1) * P, :], outS[:])

    # ---------- sparse stage 3: compact expert outputs ----------
    eoC_d = [dram.tile([CAP, H], bf16, name=f"eoC{j}") for j in range(ELOC)]
    for j in range(ELOC):
        for ct in range(CT):
            ps3 = [ps.tile([P, NCH], f32, tag="ps", name="ps3")
                   for _ in range(HC)]
            for i in range(IT):
                for hc in range(HC):
                    nc.tensor.matmul(
                        ps3[hc][:],
                        aC[j][i][:, ct * P:(ct + 1) * P],
                        dwt[j][i][hc][:],
                        start=(i == 0), stop=(i == IT - 1))
            outC = sp.tile([P, H], bf16, tag="outC", name="outC")
            for hc in range(HC):
                nc.vector.tensor_copy(outC[:, hc * NCH:(hc + 1) * NCH],
                                      ps3[hc][:])
            nc.sync.dma_start(eoC_d[j][ct * P:(ct + 1) * P, :], outC[:])
    if PHASE < 7:
        return

    # ---------- combine: gather expert rows + shared, write rs_in ------
    rs_in = dram.tile([T, H], bf16)
    rs_out = dram.tile([TLOC, H], bf16)
    for t in range(TT):
        g_sb = []
        for j in range(ELOC):
            g = sp.tile([P, H], bf16, tag=f"g{j}", name=f"g{j}")
            nc.gpsimd.indirect_dma_start(
                out=g[:], out_offset=None,
                in_=eoC_d[j][:],
                in_offset=bass.IndirectOffsetOnAxis(
                    ap=posi[:, t * ELOC + j:t * ELOC + j + 1], axis=0))
            g_sb.append(g)
        shl = sp.tile([P, H], bf16, tag="shl", name="shl")
        nc.sync.dma_start(shl[:], shpart_d[t * P:(t + 1) * P, :])
        prt = sp.tile([P, H], bf16, tag="prt", name="prt")
        nc.vector.tensor_tensor(prt[:], g_sb[0][:], g_sb[1][:],
                                op=mybir.AluOpType.add)
        nc.vector.tensor_tensor(prt[:], prt[:], shl[:],
                                op=mybir.AluOpType.add)
        nc.sync.dma_start(rs_in[t * P:(t + 1) * P, :], prt[:])
    if PHASE < 8:
        return

    # ---------- reduce-scatter + output ----------
    nc.gpsimd.collective_compute(
        "ReduceScatter", mybir.AluOpType.add,
        replica_groups=[list(range(N_CORES))],
        ins=[rs_in[:]], outs=[rs_out[:]])
    nc.gpsimd.dma_start(out_d[:], rs_out[:])  # bf16 -> f32 cast


def build_nc():
    nc = bacc.Bacc("TRN2", target_bir_lowering=False, debug=False,
                   num_devices=N_CORES)
    xT_d = nc.dram_tensor("xT", [HT, P, T], f32, kind="ExternalInput")
    gwr_d = nc.dram_tensor("gwr", [HT, P, E], f32, kind="ExternalInput")
    cb_d = nc.dram_tensor("cb", [1, E], f32, kind="ExternalInput")
    sel_d = nc.dram_tensor("sel", [ELOC, E, P], f32, kind="ExternalInput")
    w1_d = nc.dram_tensor("w1", [ELOC, 2, IT, HT, P, P], bf16,
                          kind="ExternalInput")
    d3_d = nc.dram_tensor("d3", [ELOC, IT, HC, P, NCH], bf16,
                          kind="ExternalInput")
    sg1_d = nc.dram_tensor("sg1", [2, HT, P, SIL], bf16,
                           kind="ExternalInput")
    sd3_d = nc.dram_tensor("sd3", [HC, P, NCH], bf16, kind="ExternalInput")
    x_bf_d = nc.dram_tensor("x_bf", [T, H], bf16, kind="ExternalInput")
    out_d = nc.dram_tensor("out_loc", [TLOC, H], f32, kind="ExternalOutput")
    lg_d = nc.dram_tensor("logits", [T, E], f32, kind="ExternalOutput")
    tensors = (xT_d, gwr_d, cb_d, sel_d, w1_d, d3_d, sg1_d, sd3_d,
               out_d, lg_d, x_bf_d)

    with tile.TileContext(nc) as tc:
        with (
            tc.tile_pool(name="persist", bufs=1) as pp,
            tc.tile_pool(name="stream", bufs=2) as sp,
            tc.tile_pool(name="wstream", bufs=3) as wp,
            tc.tile_pool(name="ps", bufs=6, space="PSUM") as ps,
            tc.tile_pool(name="dram", bufs=1, space="DRAM") as dram,
        ):
            _emit(nc, tc, pp, sp, wp, ps, dram, tensors)

    nc.compile()
    return nc


def pack_inputs(hidden_states, gate_weight, corr_bias, gate_w, up_w, down_w,
                shared_gate_w, shared_up_w, shared_down_w):
    x = np.asarray(hidden_states, np.float32).reshape(T, H)
    xT = np.ascontiguousarray(x.T).reshape(HT, P, T)
    gwr = np.ascontiguousarray(np.asarray(gate_weight, np.float32)
                               .reshape(HT, P, E))
    cb = np.asarray(corr_bias, np.float32).reshape(1, E)

    gate_w = np.asarray(gate_w, np.float32)
    up_w = np.asarray(up_w, np.float32)
    down_w = np.asarray(down_w, np.float32)
    sgw = np.asarray(shared_gate_w, np.float32)
    suw = np.asarray(shared_up_w, np.float32)
    sdw = np.asarray(shared_down_w, np.float32)

    in_maps = []
    for c in range(N_CORES):
        es = [ELOC * c + j for j in range(ELOC)]
        # w1 [ELOC, 2, IT, HT, P, P]: [j,pj,i,h,a,b] = W[h*128+a, i*128+b]
        w1 = np.empty((ELOC, 2, IT, HT, P, P), bfnp)
        for j, e in enumerate(es):
            for pj, W in ((0, gate_w[e]), (1, up_w[e])):
                w1[j, pj] = (W.reshape(HT, P, IT, P).transpose(2, 0, 1, 3)
                             .astype(bfnp))
        # d3 [ELOC, IT, HC, P, NCH]: [j,i,hc,a,b] = dw[i*128+a, hc*512+b]
        d3 = np.empty((ELOC, IT, HC, P, NCH), bfnp)
        for j, e in enumerate(es):
            d3[j] = (down_w[e].reshape(IT, P, HC, NCH).transpose(0, 2, 1, 3)
                     .astype(bfnp))
        # shared: si slice [c*SIL:(c+1)*SIL]
        ssl = slice(c * SIL, (c + 1) * SIL)
        sg1 = np.empty((2, HT, P, SIL), bfnp)
        sg1[0] = sgw[:, ssl].reshape(HT, P, SIL).astype(bfnp)
        sg1[1] = suw[:, ssl].reshape(HT, P, SIL).astype(bfnp)
        sd3 = np.ascontiguousarray(
            sdw[ssl].reshape(P, HC, NCH).transpose(1, 0, 2).astype(bfnp))
        # sel [ELOC, E, P]: one-hot row -> broadcast matmul operand
        sel = np.zeros((ELOC, E, P), np.float32)
        for j, e in enumerate(es):
            sel[j, e, :] = 1.0
        in_maps.append({
            "xT": xT, "gwr": gwr, "cb": cb, "sel": sel,
            "x_bf": x.astype(bfnp),
            "w1": np.ascontiguousarray(w1),
            "d3": np.ascontiguousarray(d3),
            "sg1": np.ascontiguousarray(sg1),
            "sd3": sd3,
        })
    return in_maps


_NC_CACHE = []


def _get_nc():
    if not _NC_CACHE:
        _NC_CACHE.append(build_nc())
    return _NC_CACHE[0]


def run(in_maps, trace=False):
    nc = _get_nc()
    return bass_utils.run_bass_kernel_spmd(
        nc, in_maps, core_ids=list(range(N_CORES)), trace=trace)


def kernel(**inputs):
    in_maps = pack_inputs(**inputs)
    res = run(in_maps, trace=False)
    out = np.concatenate(
        [res.results[c]["out_loc"] for c in range(N_CORES)], axis=0)
    logits = res.results[0]["logits"]
    return out.reshape(B, S, H).astype(np.float32), logits.astype(np.float32)


# revision 31
# speedup vs baseline: 1.5478x; 1.5478x over previous
"""Ernie4.5 MoE sparse block on 8 trn2 NeuronCores.

Strategy (expert-parallel, dense over tokens):
  - 16 experts sharded 2/core; each core computes its 2 experts for ALL
    2048 tokens in bf16, with router combine-weights folded into the
    activations before the down-projection (so unselected experts
    contribute exactly 0).
  - Shared-expert MLP is tensor-parallel along its intermediate dim
    (128 slice / core) and accumulated in the same PSUM group as the
    expert down-projections.
  - Router (logits, softmax-free top-4, combine weights) is computed in
    fp32 on every core (cheap, and fp32 selection is required: bf16
    logits flip ~5% of top-4 picks).
  - One bf16 ReduceScatter sums the 8 per-core partials; core r ends up
    with tokens [256r:256(r+1)] which the host concatenates.

Host-side work is marshalling only: reshape/transpose/cast of inputs,
concatenation of outputs. All arithmetic runs on device.
"""
import os
import sys

sys.path.insert(0, "/opt/trn_rl_repo")

import numpy as np
import ml_dtypes

from concourse import bacc, mybir, tile
from concourse import bass_utils
from concourse import masks

P = 128
B, S, H = 2, 1024, 1024
T = B * S                    # 2048 tokens
E, I, K = 16, 512, 4
SI = 1024                    # shared intermediate (512*2)
N_CORES = 8
ELOC = E // N_CORES          # 2 experts per core
SIL = SI // N_CORES          # 128 shared-intermediate slice per core
HT = H // P                  # 8 h-tiles
IT = I // P                  # 4 i-tiles
TT = T // P                  # 16 token-tiles
NCH = 512                    # matmul moving free dim
TC = T // NCH                # 4 token chunks
HC = H // NCH                # 2 h chunks
TLOC = T // N_CORES          # 256 tokens per core after RS

f32 = mybir.dt.float32
bf16 = mybir.dt.bfloat16
bfnp = ml_dtypes.bfloat16

PHASE = int(os.environ.get("KERNEL_PHASE", "99"))
DEBUG = int(os.environ.get("KERNEL_DEBUG", "0"))


def _emit(nc, tc, pp, sp, wp, ps, dram, tensors):
    (xT_d, gwr_d, cb_d, sel_d, w1_d, d3_d, sg1_d, sd3_d, out_d, lg_d) = tensors

    # ---------- constants ----------
    ident = pp.tile([P, P], f32, tag="ident")
    masks.make_identity(nc, ident[:])

    cbb = pp.tile([P, E], f32, tag="cbb")       # corr_bias bcast
    ones1 = pp.tile([1, P], f32, tag="ones1")
    nc.gpsimd.memset(ones1[:], 1.0)
    cb_sb = pp.tile([1, E], f32, tag="cb_sb")
    nc.sync.dma_start(cb_sb[:], cb_d[:])
    ps_cb = ps.tile([P, NCH], f32, tag="ps")
    nc.tensor.matmul(ps_cb[:, :E], ones1[:], cb_sb[:],
                     start=True, stop=True)
    nc.vector.tensor_copy(cbb[:], ps_cb[:, :E])

    sel_sb = [pp.tile([E, P], f32, tag=f"sel{j}", name=f"sel{j}")
              for j in range(ELOC)]
    for j in range(ELOC):
        nc.sync.dma_start(sel_sb[j][:], sel_d[j])

    # ---------- router matmuls (fp32) + x cast ----------
    xTb = [pp.tile([P, T], bf16, tag=f"xTb{h}", name=f"xTb{h}")
           for h in range(HT)]
    ps_lg = ps.tile([P, TT * E], f32, tag="pslg", bufs=1)
    nc.vector.memset(ps_lg[:], 0.0)
    for h in range(HT):
        xf = sp.tile([P, T], f32, tag="xf")
        nc.sync.dma_start(xf[:], xT_d[h])
        nc.scalar.activation(xTb[h][:], xf[:],
                             mybir.ActivationFunctionType.Copy)
        gr = sp.tile([P, E], f32, tag="gr")
        nc.sync.dma_start(gr[:], gwr_d[h])
        for t in range(TT):
            nc.tensor.matmul(
                ps_lg[:, t * E:(t + 1) * E],
                xf[:, t * P:(t + 1) * P],
                gr[:],
                start=False, stop=(h == HT - 1),
                skip_group_check=True)

    lg_all = pp.tile([P, TT * E], f32, tag="lg_all")
    nc.vector.tensor_copy(lg_all[:], ps_lg[:])
    # raw layout [p, t*16+e]; host reorders to [T, E]
    nc.sync.dma_start(lg_d[:], lg_all[:])
    if PHASE < 2:
        return

    # ---------- routing (softmax-free top-4 combine weights) ----------
    cw_all = pp.tile([P, TT * E], f32, tag="cw_all")
    for t in range(TT):
        Lt = lg_all[:, t * E:(t + 1) * E]
        m8 = sp.tile([P, 8], f32, tag="m8")
        nc.vector.max(m8[:], Lt)
        negm = sp.tile([P, 1], f32, tag="negm")
        nc.vector.tensor_scalar_mul(negm[:], m8[:, 0:1], -1.0)
        pun = sp.tile([P, E], f32, tag="pun")
        nc.scalar.activation(pun[:], Lt,
                             mybir.ActivationFunctionType.Exp,
                             bias=negm[:])
        ssum = sp.tile([P, 1], f32, tag="ssum")
        nc.vector.reduce_sum(ssum[:], pun[:], axis=mybir.AxisListType.X)
        # selection scores s = pun + cb*ssum (monotone-equiv to
        # softmax(logits)+cb per token)
        s_un = sp.tile([P, E], f32, tag="s_un")
        nc.vector.scalar_tensor_tensor(
            s_un[:], cbb[:], ssum[:, 0:1], pun[:],
            op0=mybir.AluOpType.mult, op1=mybir.AluOpType.add)
        s8 = sp.tile([P, 8], f32, tag="s8")
        nc.vector.max(s8[:], s_un[:])
        # rw = (s >= 4th-largest) * pun
        rw = sp.tile([P, E], f32, tag="rw")
        nc.vector.scalar_tensor_tensor(
            rw[:], s_un[:], s8[:, K - 1:K], pun[:],
            op0=mybir.AluOpType.is_ge, op1=mybir.AluOpType.mult)
        rsum = sp.tile([P, 1], f32, tag="rsum")
        nc.vector.reduce_sum(rsum[:], rw[:], axis=mybir.AxisListType.X)
        zeps = sp.tile([P, 1], f32, tag="zeps")
        nc.vector.tensor_scalar_mul(zeps[:], ssum[:], 1e-12)
        den = sp.tile([P, 1], f32, tag="den")
        nc.vector.tensor_tensor(den[:], rsum[:], zeps[:],
                                op=mybir.AluOpType.max)
        rden = sp.tile([P, 1], f32, tag="rden")
        nc.vector.reciprocal(rden[:], den[:])
        nc.vector.tensor_scalar_mul(
            cw_all[:, t * E:(t + 1) * E], rw[:], rden[:, 0:1])
    if PHASE < 3:
        return

    # ---------- stage 1 machinery ----------
    silug = [pp.tile([P, T], bf16, tag=f"silug_i{i}", name=f"silug_i{i}")
             for i in range(IT)]
    aT = [[pp.tile([P, T], bf16, tag=f"aT{j}_{i}", name=f"aT{j}_{i}")
           for i in range(IT)] for j in range(ELOC)]

    def stage1_proj(j, pj, i, out_cb):
        w8 = wp.tile([P, HT * P], bf16, tag="w8", name="w8")
        nc.sync.dma_start(w8[:], w1_d[j, pj, i])
        pst = [ps.tile([P, NCH], f32, tag="ps", name="pst")
               for _ in range(TC)]
        for h in range(HT):
            for tch in range(TC):
                nc.tensor.matmul(
                    pst[tch],
                    w8[:, h * P:(h + 1) * P],
                    xTb[h][:, tch * NCH:(tch + 1) * NCH],
                    start=(h == 0), stop=(h == HT - 1))
        for tch in range(TC):
            out_cb(tch, pst[tch])

    def gate_cb(i):
        def cb_(tch, pst):
            nc.scalar.activation(
                silug[i][:, tch * NCH:(tch + 1) * NCH], pst[:],
                mybir.ActivationFunctionType.Silu)
        return cb_

    # expert 0 gate proj first (no cwb dependency -> PE stays busy while
    # the routing DVE chain runs)
    for i in range(IT):
        stage1_proj(0, 0, i, gate_cb(i))
    if PHASE < 4:
        return

    # cw transpose -> cwT [16, T] f32
    cwT = pp.tile([E, T], f32, tag="cwT")
    for t in range(TT):
        ps_tr = ps.tile([P, P], f32, tag="ps", name="ps_tr")
        nc.tensor.transpose(ps_tr[:E, :],
                            cw_all[:, t * E:(t + 1) * E], ident[:])
        nc.vector.tensor_copy(cwT[:, t * P:(t + 1) * P], ps_tr[:E, :])
    # cwb_j [128, T] bf16 = broadcast of row (2c+j) of cwT
    cwb = [pp.tile([P, T], bf16, tag=f"cwb{j}", name=f"cwb{j}")
           for j in range(ELOC)]
    for j in range(ELOC):
        for tch in range(TC):
            ps_b = ps.tile([P, NCH], f32, tag="ps", name="ps_b")
            nc.tensor.matmul(
                ps_b[:], sel_sb[j][:],
                cwT[:, tch * NCH:(tch + 1) * NCH],
                start=True, stop=True)
            nc.vector.tensor_copy(
                cwb[j][:, tch * NCH:(tch + 1) * NCH], ps_b[:])
    if PHASE < 5:
        return

    def up_cb(j, i):
        def cb_(tch, pst):
            sl = slice(tch * NCH, (tch + 1) * NCH)
            tt = sp.tile([P, NCH], bf16, tag="tmpmul", name="tmpmul")
            nc.vector.tensor_tensor(tt[:], pst[:], silug[i][:, sl],
                                    op=mybir.AluOpType.mult)
            nc.vector.tensor_tensor(aT[j][i][:, sl], tt[:], cwb[j][:, sl],
                                    op=mybir.AluOpType.mult)
        return cb_

    if DEBUG:
        dbg_cw_d = nc.dram_tensor("dbg_cw", [P, TT * E], f32,
                                  kind="ExternalOutput")
        dbg_cwT_d = nc.dram_tensor("dbg_cwT", [E, T], f32,
                                   kind="ExternalOutput")
        dbg_cwb0_d = nc.dram_tensor("dbg_cwb0", [P, T], f32,
                                    kind="ExternalOutput")
        dbg_cwb0 = pp.tile([P, T], f32, tag="dbg1")
        nc.vector.tensor_copy(dbg_cwb0[:], cwb[0][:])
        nc.sync.dma_start(dbg_cw_d[:], cw_all[:])
        nc.sync.dma_start(dbg_cwT_d[:], cwT[:])
        nc.sync.dma_start(dbg_cwb0_d[:], dbg_cwb0[:])

    # expert 0 up proj, then expert 1 gate+up
    for i in range(IT):
        stage1_proj(0, 1, i, up_cb(0, i))
    for i in range(IT):
        stage1_proj(1, 0, i, gate_cb(i))
    for i in range(IT):
        stage1_proj(1, 1, i, up_cb(1, i))
    if DEBUG:
        dbg_aT_d = nc.dram_tensor("dbg_aT00", [P, T], f32,
                                  kind="ExternalOutput")
        dbg_aT = pp.tile([P, T], f32, tag="dbg2")
        nc.vector.tensor_copy(dbg_aT[:], aT[0][0][:])
        nc.sync.dma_start(dbg_aT_d[:], dbg_aT[:])
    if PHASE < 6:
        return

    # ---------- shared expert stage 1 (si-slice = 128) ----------
    ssg = pp.tile([P, T], bf16, tag="ssg")
    saT = pp.tile([P, T], bf16, tag="saT")
    for pj in range(2):
        w8 = wp.tile([P, HT * P], bf16, tag="w8", name="w8s")
        nc.sync.dma_start(w8[:], sg1_d[pj])
        pst = [ps.tile([P, NCH], f32, tag="ps", name="psts")
               for _ in range(TC)]
        for h in range(HT):
            for tch in range(TC):
                nc.tensor.matmul(
                    pst[tch],
                    w8[:, h * P:(h + 1) * P],
                    xTb[h][:, tch * NCH:(tch + 1) * NCH],
                    start=(h == 0), stop=(h == HT - 1))
        for tch in range(TC):
            sl = slice(tch * NCH, (tch + 1) * NCH)
            if pj == 0:
                nc.scalar.activation(ssg[:, sl], pst[tch][:],
                                     mybir.ActivationFunctionType.Silu)
            else:
                nc.vector.tensor_tensor(saT[:, sl], pst[tch][:],
                                        ssg[:, sl],
                                        op=mybir.AluOpType.mult)
    if PHASE < 7:
        return

    # ---------- stage 3: down-proj + combine into [t, h] ----------
    dwt = [[[pp.tile([P, NCH], bf16, tag=f"dw{j}_{i}_{hc}",
                     name=f"dw{j}_{i}_{hc}")
             for hc in range(HC)] for i in range(IT)] for j in range(ELOC)]
    sdwt = [pp.tile([P, NCH], bf16, tag=f"sdw{hc}", name=f"sdw{hc}")
            for hc in range(HC)]  # indexed [hc] like dwt[j][i]
    for j in range(ELOC):
        for i in range(IT):
            for hc in range(HC):
                nc.sync.dma_start(dwt[j][i][hc][:], d3_d[j, i, hc])
    for hc in range(HC):
        nc.sync.dma_start(sdwt[hc][:], sd3_d[hc])

    rs_in = dram.tile([T, H], bf16)
    rs_out = dram.tile([TLOC, H], bf16)
    n_acc = ELOC * IT + 1
    for t in range(TT):
        prt = sp.tile([P, H], bf16, tag="prt")
        ps3 = [ps.tile([P, NCH], f32, tag="ps", name="ps3")
               for _ in range(HC)]
        # hc inner so each stationary aT tile is loaded once for both
        # h-chunks (halves LDWEIGHTS traffic in stage 3)
        lhs_seq = ([(aT[j][i], dwt[j][i]) for j in range(ELOC)
                    for i in range(IT)] + [(saT, sdwt)])
        for k, (lhsT_t, rhs_t) in enumerate(lhs_seq):
            for hc in range(HC):
                nc.tensor.matmul(
                    ps3[hc][:],
                    lhsT_t[:, t * P:(t + 1) * P],
                    rhs_t[hc][:],
                    start=(k == 0), stop=(k == n_acc - 1))
        for hc in range(HC):
            nc.vector.tensor_copy(prt[:, hc * NCH:(hc + 1) * NCH],
                                  ps3[hc][:])
        nc.sync.dma_start(rs_in[t * P:(t + 1) * P, :], prt[:])
    if PHASE < 8:
        return

    # ---------- reduce-scatter + output ----------
    nc.gpsimd.collective_compute(
        "ReduceScatter", mybir.AluOpType.add,
        replica_groups=[list(range(N_CORES))],
        ins=[rs_in[:]], outs=[rs_out[:]])
    nc.gpsimd.dma_start(out_d[:], rs_out[:])  # bf16 -> f32 cast


def build_nc():
    nc = bacc.Bacc("TRN2", target_bir_lowering=False, debug=False,
                   num_devices=N_CORES)
    xT_d = nc.dram_tensor("xT", [HT, P, T], f32, kind="ExternalInput")
    gwr_d = nc.dram_tensor("gwr", [HT, P, E], f32, kind="ExternalInput")
    cb_d = nc.dram_tensor("cb", [1, E], f32, kind="ExternalInput")
    sel_d = nc.dram_tensor("sel", [ELOC, E, P], f32, kind="ExternalInput")
    w1_d = nc.dram_tensor("w1", [ELOC, 2, IT, P, HT * P], bf16,
                          kind="ExternalInput")
    d3_d = nc.dram_tensor("d3", [ELOC, IT, HC, P, NCH], bf16,
                          kind="ExternalInput")
    sg1_d = nc.dram_tensor("sg1", [2, P, HT * SIL], bf16,
                           kind="ExternalInput")
    sd3_d = nc.dram_tensor("sd3", [HC, P, NCH], bf16, kind="ExternalInput")
    out_d = nc.dram_tensor("out_loc", [TLOC, H], f32, kind="ExternalOutput")
    lg_d = nc.dram_tensor("logits", [P, TT * E], f32,
                          kind="ExternalOutput")
    tensors = (xT_d, gwr_d, cb_d, sel_d, w1_d, d3_d, sg1_d, sd3_d,
               out_d, lg_d)

    with tile.TileContext(nc) as tc:
        with (
            tc.tile_pool(name="persist", bufs=1) as pp,
            tc.tile_pool(name="stream", bufs=2) as sp,
            tc.tile_pool(name="wstream", bufs=3) as wp,
            tc.tile_pool(name="ps", bufs=6, space="PSUM") as ps,
            tc.tile_pool(name="dram", bufs=1, space="DRAM") as dram,
        ):
            _emit(nc, tc, pp, sp, wp, ps, dram, tensors)

    nc.compile()
    return nc


def pack_inputs(hidden_states, gate_weight, corr_bias, gate_w, up_w, down_w,
                shared_gate_w, shared_up_w, shared_down_w):
    x = np.asarray(hidden_states, np.float32).reshape(T, H)
    xT = np.ascontiguousarray(x.T).reshape(HT, P, T)
    gwr = np.ascontiguousarray(np.asarray(gate_weight, np.float32)
                               .reshape(HT, P, E))
    cb = np.asarray(corr_bias, np.float32).reshape(1, E)

    gate_w = np.asarray(gate_w, np.float32)
    up_w = np.asarray(up_w, np.float32)
    down_w = np.asarray(down_w, np.float32)
    sgw = np.asarray(shared_gate_w, np.float32)
    suw = np.asarray(shared_up_w, np.float32)
    sdw = np.asarray(shared_down_w, np.float32)

    in_maps = []
    for c in range(N_CORES):
        es = [ELOC * c + j for j in range(ELOC)]
        # w1 [ELOC, 2, IT, HT, P, P]: [j,pj,i,h,a,b] = W[h*128+a, i*128+b]
        w1 = np.empty((ELOC, 2, IT, P, HT * P), bfnp)
        for j, e in enumerate(es):
            for pj, W in ((0, gate_w[e]), (1, up_w[e])):
                w1[j, pj] = (W.reshape(HT, P, IT, P).transpose(2, 1, 0, 3)
                             .reshape(IT, P, HT * P).astype(bfnp))
        # d3 [ELOC, IT, HC, P, NCH]: [j,i,hc,a,b] = dw[i*128+a, hc*512+b]
        d3 = np.empty((ELOC, IT, HC, P, NCH), bfnp)
        for j, e in enumerate(es):
            d3[j] = (down_w[e].reshape(IT, P, HC, NCH).transpose(0, 2, 1, 3)
                     .astype(bfnp))
        # shared: si slice [c*SIL:(c+1)*SIL]
        ssl = slice(c * SIL, (c + 1) * SIL)
        sg1 = np.empty((2, P, HT * SIL), bfnp)
        sg1[0] = (sgw[:, ssl].reshape(HT, P, SIL).transpose(1, 0, 2)
                  .reshape(P, HT * SIL).astype(bfnp))
        sg1[1] = (suw[:, ssl].reshape(HT, P, SIL).transpose(1, 0, 2)
                  .reshape(P, HT * SIL).astype(bfnp))
        sd3 = np.ascontiguousarray(
            sdw[ssl].reshape(P, HC, NCH).transpose(1, 0, 2).astype(bfnp))
        # sel [ELOC, E, P]: one-hot row -> broadcast matmul operand
        sel = np.zeros((ELOC, E, P), np.float32)
        for j, e in enumerate(es):
            sel[j, e, :] = 1.0
        in_maps.append({
            "xT": xT, "gwr": gwr, "cb": cb, "sel": sel,
            "w1": np.ascontiguousarray(w1),
            "d3": np.ascontiguousarray(d3),
            "sg1": np.ascontiguousarray(sg1),
            "sd3": sd3,
        })
    return in_maps


_NC_CACHE = []


def _get_nc():
    if not _NC_CACHE:
        _NC_CACHE.append(build_nc())
    return _NC_CACHE[0]


def run(in_maps, trace=False):
    nc = _get_nc()
    return bass_utils.run_bass_kernel_spmd(
        nc, in_maps, core_ids=list(range(N_CORES)), trace=trace)


def kernel(**inputs):
    in_maps = pack_inputs(**inputs)
    res = run(in_maps, trace=False)
    out = np.concatenate(
        [res.results[c]["out_loc"] for c in range(N_CORES)], axis=0)
    lg_raw = res.results[0]["logits"]          # [128, TT*16]
    logits = (lg_raw.reshape(P, TT, E).transpose(1, 0, 2)
              .reshape(T, E))
    return out.reshape(B, S, H).astype(np.float32), logits.astype(np.float32)


# revision 32
# speedup vs baseline: 1.6527x; 1.0677x over previous
"""Ernie4.5 MoE sparse block on 8 trn2 NeuronCores.

Strategy (expert-parallel, dense over tokens):
  - 16 experts sharded 2/core; each core computes its 2 experts for ALL
    2048 tokens in bf16, with router combine-weights folded into the
    activations before the down-projection (so unselected experts
    contribute exactly 0).
  - Shared-expert MLP is tensor-parallel along its intermediate dim
    (128 slice / core) and accumulated in the same PSUM group as the
    expert down-projections.
  - Router (logits, softmax-free top-4, combine weights) is computed in
    fp32 on every core (cheap, and fp32 selection is required: bf16
    logits flip ~5% of top-4 picks).
  - One bf16 ReduceScatter sums the 8 per-core partials; core r ends up
    with tokens [256r:256(r+1)] which the host concatenates.

Host-side work is marshalling only: reshape/transpose/cast of inputs,
concatenation of outputs. All arithmetic runs on device.
"""
import os
import sys

sys.path.insert(0, "/opt/trn_rl_repo")

import numpy as np
import ml_dtypes

from concourse import bacc, mybir, tile
from concourse import bass_utils
from concourse import masks

P = 128
B, S, H = 2, 1024, 1024
T = B * S                    # 2048 tokens
E, I, K = 16, 512, 4
SI = 1024                    # shared intermediate (512*2)
N_CORES = 8
ELOC = E // N_CORES          # 2 experts per core
SIL = SI // N_CORES          # 128 shared-intermediate slice per core
HT = H // P                  # 8 h-tiles
IT = I // P                  # 4 i-tiles
TT = T // P                  # 16 token-tiles
NCH = 512                    # matmul moving free dim
TC = T // NCH                # 4 token chunks
HC = H // NCH                # 2 h chunks
TLOC = T // N_CORES          # 256 tokens per core after RS

f32 = mybir.dt.float32
bf16 = mybir.dt.bfloat16
bfnp = ml_dtypes.bfloat16

PHASE = int(os.environ.get("KERNEL_PHASE", "99"))
DEBUG = int(os.environ.get("KERNEL_DEBUG", "0"))


def _emit(nc, tc, pp, sp, wp, ps, dram, tensors):
    (xT_d, gwr_d, cb_d, sel_d, w1_d, d3_d, sg1_d, sd3_d, out_d, lg_d) = tensors

    # ---------- constants ----------
    ident = pp.tile([P, P], f32, tag="ident")
    masks.make_identity(nc, ident[:])

    cbb = pp.tile([P, E], f32, tag="cbb")       # corr_bias bcast
    ones1 = pp.tile([1, P], f32, tag="ones1")
    nc.gpsimd.memset(ones1[:], 1.0)
    cb_sb = pp.tile([1, E], f32, tag="cb_sb")
    nc.sync.dma_start(cb_sb[:], cb_d[:])
    ps_cb = ps.tile([P, NCH], f32, tag="ps")
    nc.tensor.matmul(ps_cb[:, :E], ones1[:], cb_sb[:],
                     start=True, stop=True)
    nc.vector.tensor_copy(cbb[:], ps_cb[:, :E])

    sel_sb = [pp.tile([E, P], f32, tag=f"sel{j}", name=f"sel{j}")
              for j in range(ELOC)]
    for j in range(ELOC):
        nc.sync.dma_start(sel_sb[j][:], sel_d[j])

    # ---------- router matmuls (fp32) + x cast ----------
    xTb = [pp.tile([P, T], bf16, tag=f"xTb{h}", name=f"xTb{h}")
           for h in range(HT)]
    ps_lg = ps.tile([P, TT * E], f32, tag="pslg", bufs=1)
    nc.vector.memset(ps_lg[:], 0.0)
    for h in range(HT):
        xf = sp.tile([P, T], f32, tag="xf")
        nc.sync.dma_start(xf[:], xT_d[h])
        nc.scalar.activation(xTb[h][:], xf[:],
                             mybir.ActivationFunctionType.Copy)
        gr = sp.tile([P, E], f32, tag="gr")
        nc.sync.dma_start(gr[:], gwr_d[h])
        for t in range(TT):
            nc.tensor.matmul(
                ps_lg[:, t * E:(t + 1) * E],
                xf[:, t * P:(t + 1) * P],
                gr[:],
                start=False, stop=(h == HT - 1),
                skip_group_check=True)

    lg_all = pp.tile([P, TT * E], f32, tag="lg_all")
    nc.vector.tensor_copy(lg_all[:], ps_lg[:])
    # raw layout [p, t*16+e]; host reorders to [T, E]
    nc.sync.dma_start(lg_d[:], lg_all[:])
    if PHASE < 2:
        return

    # ---------- routing (softmax-free top-4 combine weights) ----------
    cw_all = pp.tile([P, TT * E], f32, tag="cw_all")
    for t in range(TT):
        Lt = lg_all[:, t * E:(t + 1) * E]
        m8 = sp.tile([P, 8], f32, tag="m8")
        nc.vector.max(m8[:], Lt)
        negm = sp.tile([P, 1], f32, tag="negm")
        nc.vector.tensor_scalar_mul(negm[:], m8[:, 0:1], -1.0)
        pun = sp.tile([P, E], f32, tag="pun")
        nc.scalar.activation(pun[:], Lt,
                             mybir.ActivationFunctionType.Exp,
                             bias=negm[:])
        ssum = sp.tile([P, 1], f32, tag="ssum")
        nc.vector.reduce_sum(ssum[:], pun[:], axis=mybir.AxisListType.X)
        # selection scores s = pun + cb*ssum (monotone-equiv to
        # softmax(logits)+cb per token)
        s_un = sp.tile([P, E], f32, tag="s_un")
        nc.vector.scalar_tensor_tensor(
            s_un[:], cbb[:], ssum[:, 0:1], pun[:],
            op0=mybir.AluOpType.mult, op1=mybir.AluOpType.add)
        s8 = sp.tile([P, 8], f32, tag="s8")
        nc.vector.max(s8[:], s_un[:])
        # rw = (s >= 4th-largest) * pun
        rw = sp.tile([P, E], f32, tag="rw")
        nc.vector.scalar_tensor_tensor(
            rw[:], s_un[:], s8[:, K - 1:K], pun[:],
            op0=mybir.AluOpType.is_ge, op1=mybir.AluOpType.mult)
        rsum = sp.tile([P, 1], f32, tag="rsum")
        nc.vector.reduce_sum(rsum[:], rw[:], axis=mybir.AxisListType.X)
        zeps = sp.tile([P, 1], f32, tag="zeps")
        nc.vector.tensor_scalar_mul(zeps[:], ssum[:], 1e-12)
        den = sp.tile([P, 1], f32, tag="den")
        nc.vector.tensor_tensor(den[:], rsum[:], zeps[:],
                                op=mybir.AluOpType.max)
        rden = sp.tile([P, 1], f32, tag="rden")
        nc.vector.reciprocal(rden[:], den[:])
        nc.vector.tensor_scalar_mul(
            cw_all[:, t * E:(t + 1) * E], rw[:], rden[:, 0:1])
    if PHASE < 3:
        return

    # ---------- stage 1 machinery ----------
    silug = [pp.tile([P, T], bf16, tag=f"silug_i{i}", name=f"silug_i{i}")
             for i in range(IT)]
    aT = [[pp.tile([P, T], bf16, tag=f"aT{j}_{i}", name=f"aT{j}_{i}")
           for i in range(IT)] for j in range(ELOC)]

    def stage1_proj(j, pj, i, out_cb):
        w8 = wp.tile([P, HT * P], bf16, tag="w8", name="w8")
        nc.sync.dma_start(w8[:], w1_d[j, pj, i])
        pst = [ps.tile([P, NCH], f32, tag="ps", name="pst")
               for _ in range(TC)]
        for h in range(HT):
            for tch in range(TC):
                nc.tensor.matmul(
                    pst[tch],
                    w8[:, h * P:(h + 1) * P],
                    xTb[h][:, tch * NCH:(tch + 1) * NCH],
                    start=(h == 0), stop=(h == HT - 1))
        for tch in range(TC):
            out_cb(tch, pst[tch])

    def gate_cb(i):
        def cb_(tch, pst):
            nc.scalar.activation(
                silug[i][:, tch * NCH:(tch + 1) * NCH], pst[:],
                mybir.ActivationFunctionType.Silu)
        return cb_

    # expert 0 gate proj first (no cwb dependency -> PE stays busy while
    # the routing DVE chain runs)
    for i in range(IT):
        stage1_proj(0, 0, i, gate_cb(i))
    if PHASE < 4:
        return

    # cw transpose -> cwT [16, T] f32
    cwT = pp.tile([E, T], f32, tag="cwT")
    for t in range(TT):
        ps_tr = ps.tile([P, P], f32, tag="ps", name="ps_tr")
        nc.tensor.transpose(ps_tr[:E, :],
                            cw_all[:, t * E:(t + 1) * E], ident[:])
        nc.vector.tensor_copy(cwT[:, t * P:(t + 1) * P], ps_tr[:E, :])
    # cwb_j [128, T] bf16 = broadcast of row (2c+j) of cwT
    cwb = [pp.tile([P, T], bf16, tag=f"cwb{j}", name=f"cwb{j}")
           for j in range(ELOC)]
    for j in range(ELOC):
        for tch in range(TC):
            ps_b = ps.tile([P, NCH], f32, tag="ps", name="ps_b")
            nc.tensor.matmul(
                ps_b[:], sel_sb[j][:],
                cwT[:, tch * NCH:(tch + 1) * NCH],
                start=True, stop=True)
            nc.vector.tensor_copy(
                cwb[j][:, tch * NCH:(tch + 1) * NCH], ps_b[:])
    if PHASE < 5:
        return

    def up_cb(j, i):
        def cb_(tch, pst):
            sl = slice(tch * NCH, (tch + 1) * NCH)
            tt = sp.tile([P, NCH], bf16, tag="tmpmul", name="tmpmul")
            nc.vector.tensor_tensor(tt[:], pst[:], silug[i][:, sl],
                                    op=mybir.AluOpType.mult)
            nc.vector.tensor_tensor(aT[j][i][:, sl], tt[:], cwb[j][:, sl],
                                    op=mybir.AluOpType.mult)
        return cb_

    if DEBUG:
        dbg_cw_d = nc.dram_tensor("dbg_cw", [P, TT * E], f32,
                                  kind="ExternalOutput")
        dbg_cwT_d = nc.dram_tensor("dbg_cwT", [E, T], f32,
                                   kind="ExternalOutput")
        dbg_cwb0_d = nc.dram_tensor("dbg_cwb0", [P, T], f32,
                                    kind="ExternalOutput")
        dbg_cwb0 = pp.tile([P, T], f32, tag="dbg1")
        nc.vector.tensor_copy(dbg_cwb0[:], cwb[0][:])
        nc.sync.dma_start(dbg_cw_d[:], cw_all[:])
        nc.sync.dma_start(dbg_cwT_d[:], cwT[:])
        nc.sync.dma_start(dbg_cwb0_d[:], dbg_cwb0[:])

    # expert 0 up proj, then expert 1 gate+up
    for i in range(IT):
        stage1_proj(0, 1, i, up_cb(0, i))
    for i in range(IT):
        stage1_proj(1, 0, i, gate_cb(i))
    for i in range(IT):
        stage1_proj(1, 1, i, up_cb(1, i))
    if DEBUG:
        dbg_aT_d = nc.dram_tensor("dbg_aT00", [P, T], f32,
                                  kind="ExternalOutput")
        dbg_aT = pp.tile([P, T], f32, tag="dbg2")
        nc.vector.tensor_copy(dbg_aT[:], aT[0][0][:])
        nc.sync.dma_start(dbg_aT_d[:], dbg_aT[:])
    if PHASE < 6:
        return

    # ---------- shared expert stage 1 (si-slice = 128) ----------
    ssg = pp.tile([P, T], bf16, tag="ssg")
    saT = pp.tile([P, T], bf16, tag="saT")
    for pj in range(2):
        w8 = wp.tile([P, HT * P], bf16, tag="w8", name="w8s")
        nc.sync.dma_start(w8[:], sg1_d[pj])
        pst = [ps.tile([P, NCH], f32, tag="ps", name="psts")
               for _ in range(TC)]
        for h in range(HT):
            for tch in range(TC):
                nc.tensor.matmul(
                    pst[tch],
                    w8[:, h * P:(h + 1) * P],
                    xTb[h][:, tch * NCH:(tch + 1) * NCH],
                    start=(h == 0), stop=(h == HT - 1))
        for tch in range(TC):
            sl = slice(tch * NCH, (tch + 1) * NCH)
            if pj == 0:
                nc.scalar.activation(ssg[:, sl], pst[tch][:],
                                     mybir.ActivationFunctionType.Silu)
            else:
                nc.vector.tensor_tensor(saT[:, sl], pst[tch][:],
                                        ssg[:, sl],
                                        op=mybir.AluOpType.mult)
    if PHASE < 7:
        return

    # ---------- stage 3: down-proj + combine into [t, h] ----------
    dwt = [[[pp.tile([P, NCH], bf16, tag=f"dw{j}_{i}_{hc}",
                     name=f"dw{j}_{i}_{hc}")
             for hc in range(HC)] for i in range(IT)] for j in range(ELOC)]
    sdwt = [pp.tile([P, NCH], bf16, tag=f"sdw{hc}", name=f"sdw{hc}")
            for hc in range(HC)]  # indexed [hc] like dwt[j][i]
    for j in range(ELOC):
        for i in range(IT):
            for hc in range(HC):
                nc.sync.dma_start(dwt[j][i][hc][:], d3_d[j, i, hc])
    for hc in range(HC):
        nc.sync.dma_start(sdwt[hc][:], sd3_d[hc])

    rs_in = dram.tile([T, H], bf16)
    rs_out = dram.tile([2, TLOC // 2, H], bf16)
    n_acc = ELOC * IT + 1
    for t in range(TT):
        prt = sp.tile([P, H], bf16, tag="prt")
        ps3 = [ps.tile([P, NCH], f32, tag="ps", name="ps3")
               for _ in range(HC)]
        # hc inner so each stationary aT tile is loaded once for both
        # h-chunks (halves LDWEIGHTS traffic in stage 3)
        lhs_seq = ([(aT[j][i], dwt[j][i]) for j in range(ELOC)
                    for i in range(IT)] + [(saT, sdwt)])
        for k, (lhsT_t, rhs_t) in enumerate(lhs_seq):
            for hc in range(HC):
                nc.tensor.matmul(
                    ps3[hc][:],
                    lhsT_t[:, t * P:(t + 1) * P],
                    rhs_t[hc][:],
                    start=(k == 0), stop=(k == n_acc - 1))
        for hc in range(HC):
            nc.vector.tensor_copy(prt[:, hc * NCH:(hc + 1) * NCH],
                                  ps3[hc][:])
        nc.sync.dma_start(rs_in[t * P:(t + 1) * P, :], prt[:])
        # launch the first-half reduce-scatter as soon as rows 0..1023
        # are written so it overlaps the second half of stage 3
        if t == TT // 2 - 1:
            nc.gpsimd.collective_compute(
                "ReduceScatter", mybir.AluOpType.add,
                replica_groups=[list(range(N_CORES))],
                ins=[rs_in[0:T // 2, :]], outs=[rs_out[0]])
    if PHASE < 8:
        return

    # ---------- second-half reduce-scatter + output ----------
    nc.gpsimd.collective_compute(
        "ReduceScatter", mybir.AluOpType.add,
        replica_groups=[list(range(N_CORES))],
        ins=[rs_in[T // 2:, :]], outs=[rs_out[1]])
    nc.gpsimd.dma_start(out_d[:], rs_out[:])  # bf16 -> f32 cast


def build_nc():
    nc = bacc.Bacc("TRN2", target_bir_lowering=False, debug=False,
                   num_devices=N_CORES)
    xT_d = nc.dram_tensor("xT", [HT, P, T], f32, kind="ExternalInput")
    gwr_d = nc.dram_tensor("gwr", [HT, P, E], f32, kind="ExternalInput")
    cb_d = nc.dram_tensor("cb", [1, E], f32, kind="ExternalInput")
    sel_d = nc.dram_tensor("sel", [ELOC, E, P], f32, kind="ExternalInput")
    w1_d = nc.dram_tensor("w1", [ELOC, 2, IT, P, HT * P], bf16,
                          kind="ExternalInput")
    d3_d = nc.dram_tensor("d3", [ELOC, IT, HC, P, NCH], bf16,
                          kind="ExternalInput")
    sg1_d = nc.dram_tensor("sg1", [2, P, HT * SIL], bf16,
                           kind="ExternalInput")
    sd3_d = nc.dram_tensor("sd3", [HC, P, NCH], bf16, kind="ExternalInput")
    out_d = nc.dram_tensor("out_loc", [TLOC, H], f32, kind="ExternalOutput")
    lg_d = nc.dram_tensor("logits", [P, TT * E], f32,
                          kind="ExternalOutput")
    tensors = (xT_d, gwr_d, cb_d, sel_d, w1_d, d3_d, sg1_d, sd3_d,
               out_d, lg_d)

    with tile.TileContext(nc) as tc:
        with (
            tc.tile_pool(name="persist", bufs=1) as pp,
            tc.tile_pool(name="stream", bufs=2) as sp,
            tc.tile_pool(name="wstream", bufs=3) as wp,
            tc.tile_pool(name="ps", bufs=6, space="PSUM") as ps,
            tc.tile_pool(name="dram", bufs=1, space="DRAM") as dram,
        ):
            _emit(nc, tc, pp, sp, wp, ps, dram, tensors)

    nc.compile()
    return nc


def pack_inputs(hidden_states, gate_weight, corr_bias, gate_w, up_w, down_w,
                shared_gate_w, shared_up_w, shared_down_w):
    x = np.asarray(hidden_states, np.float32).reshape(T, H)
    xT = np.ascontiguousarray(x.T).reshape(HT, P, T)
    gwr = np.ascontiguousarray(np.asarray(gate_weight, np.float32)
                               .reshape(HT, P, E))
    cb = np.asarray(corr_bias, np.float32).reshape(1, E)

    gate_w = np.asarray(gate_w, np.float32)
    up_w = np.asarray(up_w, np.float32)
    down_w = np.asarray(down_w, np.float32)
    sgw = np.asarray(shared_gate_w, np.float32)
    suw = np.asarray(shared_up_w, np.float32)
    sdw = np.asarray(shared_down_w, np.float32)

    in_maps = []
    for c in range(N_CORES):
        es = [ELOC * c + j for j in range(ELOC)]
        # w1 [ELOC, 2, IT, HT, P, P]: [j,pj,i,h,a,b] = W[h*128+a, i*128+b]
        w1 = np.empty((ELOC, 2, IT, P, HT * P), bfnp)
        for j, e in enumerate(es):
            for pj, W in ((0, gate_w[e]), (1, up_w[e])):
                w1[j, pj] = (W.reshape(HT, P, IT, P).transpose(2, 1, 0, 3)
                             .reshape(IT, P, HT * P).astype(bfnp))
        # d3 [ELOC, IT, HC, P, NCH]: [j,i,hc,a,b] = dw[i*128+a, hc*512+b]
        d3 = np.empty((ELOC, IT, HC, P, NCH), bfnp)
        for j, e in enumerate(es):
            d3[j] = (down_w[e].reshape(IT, P, HC, NCH).transpose(0, 2, 1, 3)
                     .astype(bfnp))
        # shared: si slice [c*SIL:(c+1)*SIL]
        ssl = slice(c * SIL, (c + 1) * SIL)
        sg1 = np.empty((2, P, HT * SIL), bfnp)
        sg1[0] = (sgw[:, ssl].reshape(HT, P, SIL).transpose(1, 0, 2)
                  .reshape(P, HT * SIL).astype(bfnp))
        sg1[1] = (suw[:, ssl].reshape(HT, P, SIL).transpose(1, 0, 2)
                  .reshape(P, HT * SIL).astype(bfnp))
        sd3 = np.ascontiguousarray(
            sdw[ssl].reshape(P, HC, NCH).transpose(1, 0, 2).astype(bfnp))
        # sel [ELOC, E, P]: one-hot row -> broadcast matmul operand
        sel = np.zeros((ELOC, E, P), np.float32)
        for j, e in enumerate(es):
            sel[j, e, :] = 1.0
        in_maps.append({
            "xT": xT, "gwr": gwr, "cb": cb, "sel": sel,
            "w1": np.ascontiguousarray(w1),
            "d3": np.ascontiguousarray(d3),
            "sg1": np.ascontiguousarray(sg1),
            "sd3": sd3,
        })
    return in_maps


_NC_CACHE = []


def _get_nc():
    if not _NC_CACHE:
        _NC_CACHE.append(build_nc())
    return _NC_CACHE[0]


def run(in_maps, trace=False):
    nc = _get_nc()
    return bass_utils.run_bass_kernel_spmd(
        nc, in_maps, core_ids=list(range(N_CORES)), trace=trace)


def kernel(**inputs):
    in_maps = pack_inputs(**inputs)
    res = run(in_maps, trace=False)
    out = np.empty((T, H), np.float32)
    half = TLOC // 2
    for c in range(N_CORES):
        o = res.results[c]["out_loc"]
        out[c * half:(c + 1) * half] = o[:half]
        out[T // 2 + c * half:T // 2 + (c + 1) * half] = o[half:]
    lg_raw = res.results[0]["logits"]          # [128, TT*16]
    logits = (lg_raw.reshape(P, TT, E).transpose(1, 0, 2)
              .reshape(T, E))
    return out.reshape(B, S, H).astype(np.float32), logits.astype(np.float32)
